# revision 12
# baseline (speedup 1.0000x reference)
"""Multi-head cross-attention (B=32, N=128, M=2048, 16 heads x 64) on 8 TRN2 cores.

Strategy: pure data-parallel over the batch dim (4 batches/core). All matmul
operands are fp16 (fp32 accumulation in PSUM); softmax skips the max-subtraction
(scores are ~N(0,1), |s|max ~ 6.5, exp stays well inside fp16 range) and the
row-sum is fused into the AV matmul as a 65th ones-column of V. Inputs are
transposed/cast on the host so every device-side matmul contracts over the
partition dim with no on-chip transposes.

Per-core device program (b = 4 batches):
  qhT  = Wq^T q^T            (heads on partitions, scale 1/8 folded into q)
  per batch:
    vh   = kv Wv             (kv tokens on partitions, + ones column per head)
    khT_c = Wk^T kv^T        (per inner chunk c of 128 = head pair)
    scoresT = khT_c^T qhT    (K=128 with the other head's q half zeroed)
    attnT = exp(scoresT)     (ACT, fp16)
    avT_h = [vh | 1]^T attnT (65 x 128; row 64 = softmax denominator)
    outT_h = avT_h / denom
    y = outT^T Wo + bo       (K=64 per head, accumulated over 16 heads)
"""
import numpy as np

NCORES = 8
B, BPC = 32, 4
N, M = 128, 2048
H, D = 16, 64
QD, KVD, INNER = 1024, 512, 1024

_cached = {}


def _build_nc():
    from contextlib import ExitStack

    import concourse.tile as tile
    from concourse import bacc, mybir

    F16 = mybir.dt.float16
    F32 = mybir.dt.float32
    AF = mybir.ActivationFunctionType

    nc = bacc.Bacc("TRN2", target_bir_lowering=False, debug=False,
                   num_devices=NCORES)
    qT_d = nc.dram_tensor("qT", [QD, BPC * N], F16, kind="ExternalInput").ap()
    kvT_d = nc.dram_tensor("kvT", [BPC, KVD, M], F16, kind="ExternalInput").ap()
    wq_d = nc.dram_tensor("Wq", [QD, INNER], F16, kind="ExternalInput").ap()
    wk_d = nc.dram_tensor("Wk", [KVD, INNER], F16, kind="ExternalInput").ap()
    wv_d = nc.dram_tensor("Wv", [KVD, INNER], F16, kind="ExternalInput").ap()
    wo_d = nc.dram_tensor("Wo", [INNER, QD], F16, kind="ExternalInput").ap()
    bo_d = nc.dram_tensor("bo", [128, QD], F32, kind="ExternalInput").ap()
    y_d = nc.dram_tensor("y", [BPC, N, QD], F32, kind="ExternalOutput").ap()

    with tile.TileContext(nc) as tc, ExitStack() as ctx:
        const = ctx.enter_context(tc.tile_pool(name="const", bufs=1))
        kvt_pool = ctx.enter_context(tc.tile_pool(name="kvt", bufs=2))
        kht_pool = ctx.enter_context(tc.tile_pool(name="kht", bufs=2))
        vh_pool = ctx.enter_context(tc.tile_pool(name="vh", bufs=1))
        attn_pool = ctx.enter_context(tc.tile_pool(name="attn", bufs=16))
        outt_pool = ctx.enter_context(tc.tile_pool(name="outt", bufs=2))
        y_pool = ctx.enter_context(tc.tile_pool(name="yp", bufs=1))
        r_pool = ctx.enter_context(tc.tile_pool(name="rp", bufs=4))
        pp = ctx.enter_context(tc.tile_pool(name="pp", bufs=3, space="PSUM"))
        scp = ctx.enter_context(tc.tile_pool(name="scp", bufs=3, space="PSUM"))
        avp = ctx.enter_context(tc.tile_pool(name="avp", bufs=2, space="PSUM"))

        # ---- constants into SBUF
        wq_sb = const.tile([128, 8 * INNER], F16)
        nc.sync.dma_start(
            wq_sb[:].rearrange("p (k n) -> p k n", k=8),
            wq_d.rearrange("(k p) n -> p k n", p=128),
        )
        wk_sb = const.tile([128, 4 * INNER], F16)
        nc.sync.dma_start(
            wk_sb[:].rearrange("p (k n) -> p k n", k=4),
            wk_d.rearrange("(k p) n -> p k n", p=128),
        )
        wv_sb = const.tile([128, 4 * INNER], F16)
        nc.sync.dma_start(
            wv_sb[:].rearrange("p (k n) -> p k n", k=4),
            wv_d.rearrange("(k p) n -> p k n", p=128),
        )
        wo_sb = const.tile([128, 8 * QD], F16)
        nc.sync.dma_start(
            wo_sb[:].rearrange("p (k n) -> p k n", k=8),
            wo_d.rearrange("(k p) n -> p k n", p=128),
        )
        bo_bc = const.tile([128, QD], F32)
        nc.sync.dma_start(bo_bc[:], bo_d[:])
        ones_sb = const.tile([1, 64], F16)
        nc.vector.memset(ones_sb[:], 1.0)
        qt_sb = const.tile([128, 8 * BPC * N], F16)
        nc.sync.dma_start(
            qt_sb[:].rearrange("p (k n) -> p k n", k=8),
            qT_d.rearrange("(k p) n -> p k n", p=128),
        )

        BN = BPC * N  # 512

        # ---- qhT projection: all 4 batches at once, chunk c = head pair.
        # Layout (c, b, hi, n): per (c, b) the two head-halves sit in adjacent
        # 128-col blocks, each with the complementary 64 partition rows zeroed,
        # so ONE K=128 N=256 scores matmul serves both heads of the pair.
        qh2 = const.tile([128, 8 * 2 * BN], F16)
        qh2v = qh2[:].rearrange("p (c b i n) -> p c b i n", c=8, b=BPC, i=2)
        nc.vector.memset(qh2v[64:128, :, :, 0, :], 0.0)
        nc.vector.memset(qh2v[0:64, :, :, 1, :], 0.0)
        for c in range(8):
            p = pp.tile([128, BN], F32, tag="pp")
            for k in range(8):
                nc.tensor.matmul(
                    p[:],
                    wq_sb[:, k * INNER + c * 128:k * INNER + (c + 1) * 128],
                    qt_sb[:, k * BN:(k + 1) * BN],
                    start=(k == 0), stop=(k == 7),
                )
            nc.vector.tensor_copy(
                qh2v[0:64, c, :, 0, :],
                p[0:64, :].rearrange("p (b n) -> p b n", b=BPC),
            )
            nc.vector.tensor_copy(
                qh2v[64:128, c, :, 1, :],
                p[64:128, :].rearrange("p (b n) -> p b n", b=BPC),
            )

        vh_sb = vh_pool.tile([128, 16 * H * 65], F16, tag="vh")
        vh4 = vh_sb[:].rearrange("p (t h d) -> p t h d", h=H, d=65)
        nc.vector.memset(vh4[:, :, :, 64:65], 1.0)

        for b in range(BPC):
            # ---- load kv^T for this batch
            kvt_sb = kvt_pool.tile([128, 4 * M], F16, tag="kvt")
            nc.sync.dma_start(
                kvt_sb[:].rearrange("p (k m) -> p k m", k=4),
                kvT_d[b].rearrange("(k p) m -> p k m", p=128),
            )

            # ---- vh = kv @ Wv, kv tokens on partitions, 65-col blocks per head
            for t in range(16):
                for n2 in range(2):
                    p = pp.tile([128, 512], F32, tag="pp")
                    for k in range(4):
                        nc.tensor.matmul(
                            p[:],
                            kvt_sb[:, k * M + t * 128:k * M + (t + 1) * 128],
                            wv_sb[:, k * INNER + n2 * 512:k * INNER + (n2 + 1) * 512],
                            start=(k == 0), stop=(k == 3),
                        )
                    nc.vector.tensor_copy(
                        vh4[:, t, n2 * 8:(n2 + 1) * 8, 0:64],
                        p[:].rearrange("p (h d) -> p h d", d=64),
                    )

            # ---- per head pair: khT chunk -> scores -> exp; AV runs one
            # pair behind so the ACT exp latency hides under PE work.
            pending = None

            def do_av(avwork):
                at_list, cc = avwork
                for hi in range(2):
                    h = 2 * cc + hi
                    p = avp.tile([128, 256], F32, tag="avp")
                    for t in range(16):
                        tg, j = t // 2, t % 2
                        nc.tensor.matmul(
                            p[0:65, 0:128],
                            vh4[:, t, h, :],
                            at_list[tg][:, j * 256 + hi * 128:j * 256 + (hi + 1) * 128],
                            start=(t == 0), stop=(t == 15),
                        )
                    r32 = r_pool.tile([1, N], F32, tag="rp32")
                    nc.vector.reciprocal(r32[:], p[64:65, 0:128])
                    r16 = r_pool.tile([1, N], F16, tag="rp")
                    nc.scalar.activation(r16[:], r32[:], AF.Copy)
                    nc.tensor.matmul(p[0:64, 128:256], ones_sb[:], r16[:],
                                     start=True, stop=True)
                    r_bc = r_pool.tile([64, N], F32, tag="rbc")
                    nc.vector.tensor_copy(r_bc[:], p[0:64, 128:256])
                    nc.vector.tensor_mul(
                        outt_sb[:, h * N:(h + 1) * N],
                        p[0:64, 0:128],
                        r_bc[:],
                    )

            outt_sb = outt_pool.tile([64, H * N], F16, tag="outt")
            for c in range(8):
                kht_sb = kht_pool.tile([128, M], F16, tag="kht")
                for n in range(4):
                    p = pp.tile([128, 512], F32, tag="pp")
                    for k in range(4):
                        nc.tensor.matmul(
                            p[:],
                            wk_sb[:, k * INNER + c * 128:k * INNER + (c + 1) * 128],
                            kvt_sb[:, k * M + n * 512:k * M + (n + 1) * 512],
                            start=(k == 0), stop=(k == 3),
                        )
                    nc.vector.tensor_copy(kht_sb[:, n * 512:(n + 1) * 512], p[:])

                at_tiles = []
                for tg in range(8):
                    sc = scp.tile([128, 512], F32, tag="scp")
                    for j in range(2):
                        t = tg * 2 + j
                        nc.tensor.matmul(
                            sc[:, j * 256:(j + 1) * 256],
                            kht_sb[:, t * 128:(t + 1) * 128],
                            qh2[:, (c * BPC + b) * 256:(c * BPC + b + 1) * 256],
                            start=True, stop=True,
                        )
                    at = attn_pool.tile([128, 512], F16, tag="attn")
                    nc.scalar.activation(at[:], sc[:], AF.Exp)
                    at_tiles.append(at)

                if pending is not None:
                    do_av(pending)
                pending = (at_tiles, c)
            do_av(pending)

            # ---- out projection: y = outT^T @ Wo + bo (K=64 per head)
            y_sb = y_pool.tile([128, QD], F32, tag="yp")
            outt2 = outt_pool.tile([128, 8 * N], F16, tag="outt2")
            ov = outt_sb[:].rearrange("p (c two n) -> p c two n", two=2, n=N)
            nc.sync.dma_start(
                outt2[0:64, :].rearrange("p (c n) -> p c n", n=N),
                ov[:, :, 0, :],
            )
            nc.sync.dma_start(
                outt2[64:128, :].rearrange("p (c n) -> p c n", n=N),
                ov[:, :, 1, :],
            )
            yp0 = pp.tile([128, 512], F32, tag="pp")
            yp1 = pp.tile([128, 512], F32, tag="pp")
            yps = [yp0, yp1]
            for c3 in range(8):
                for n2 in range(2):
                    nc.tensor.matmul(
                        yps[n2][:],
                        outt2[:, c3 * N:(c3 + 1) * N],
                        wo_sb[:, c3 * QD + n2 * 512:c3 * QD + (n2 + 1) * 512],
                        start=(c3 == 0), stop=(c3 == 7),
                    )
            for n2 in range(2):
                nc.vector.tensor_add(
                    y_sb[:, n2 * 512:(n2 + 1) * 512],
                    yps[n2][:],
                    bo_bc[:, n2 * 512:(n2 + 1) * 512],
                )
            nc.sync.dma_start(y_d[b], y_sb[:])

    nc.compile()
    return nc


def _get_nc():
    if "nc" not in _cached:
        _cached["nc"] = _build_nc()
    return _cached["nc"]


def kernel(q, kv, Wq, Wk, Wv, Wo, bo):
    from concourse.bass_utils import run_bass_kernel_spmd

    nc = _get_nc()

    wq16 = Wq.astype(np.float16)
    wk16 = Wk.astype(np.float16)
    wv16 = Wv.astype(np.float16)
    wo16 = Wo.astype(np.float16)
    bo32 = np.ascontiguousarray(
        np.broadcast_to(bo.reshape(1, QD), (128, QD)).astype(np.float32))

    scale = D ** -0.5  # 1/8, exact in fp16
    in_maps = []
    for i in range(NCORES):
        bs = slice(i * BPC, (i + 1) * BPC)
        # (BPC, N, QD) -> (QD, BPC, N) -> (QD, BPC*N), scale folded in
        qT = np.ascontiguousarray(
            np.transpose(q[bs] * scale, (2, 0, 1)).reshape(QD, BPC * N)
        ).astype(np.float16)
        kvT = np.ascontiguousarray(np.transpose(kv[bs], (0, 2, 1))).astype(
            np.float16
        )
        in_maps.append(
            {"qT": qT, "kvT": kvT, "Wq": wq16, "Wk": wk16, "Wv": wv16,
             "Wo": wo16, "bo": bo32}
        )

    _cached["in_maps"] = in_maps
    res = run_bass_kernel_spmd(nc, in_maps, list(range(NCORES)))
    out = np.concatenate([res.results[i]["y"] for i in range(NCORES)], axis=0)
    return out.astype(np.float32)


# revision 13
# speedup vs baseline: 1.0084x; 1.0084x over previous
"""Multi-head cross-attention (B=32, N=128, M=2048, 16 heads x 64) on 8 TRN2 cores.

Strategy: pure data-parallel over the batch dim (4 batches/core). All matmul
operands are fp16 (fp32 accumulation in PSUM); softmax skips the max-subtraction
(scores are ~N(0,1), |s|max ~ 6.5, exp stays well inside fp16 range) and the
row-sum is fused into the AV matmul as a 65th ones-column of V. Inputs are
transposed/cast on the host so every device-side matmul contracts over the
partition dim with no on-chip transposes.

Per-core device program (b = 4 batches):
  qhT  = Wq^T q^T            (heads on partitions, scale 1/8 folded into q)
  per batch:
    vh   = kv Wv             (kv tokens on partitions, + ones column per head)
    khT_c = Wk^T kv^T        (per inner chunk c of 128 = head pair)
    scoresT = khT_c^T qhT    (K=128 with the other head's q half zeroed)
    attnT = exp(scoresT)     (ACT, fp16)
    avT_h = [vh | 1]^T attnT (65 x 128; row 64 = softmax denominator)
    outT_h = avT_h / denom
    y = outT^T Wo + bo       (K=64 per head, accumulated over 16 heads)
"""
import numpy as np

NCORES = 8
B, BPC = 32, 4
N, M = 128, 2048
H, D = 16, 64
QD, KVD, INNER = 1024, 512, 1024

_cached = {}


def _build_nc():
    from contextlib import ExitStack

    import concourse.tile as tile
    from concourse import bacc, mybir

    F16 = mybir.dt.float16
    F32 = mybir.dt.float32
    AF = mybir.ActivationFunctionType

    nc = bacc.Bacc("TRN2", target_bir_lowering=False, debug=False,
                   num_devices=NCORES)
    qT_d = nc.dram_tensor("qT", [QD, BPC * N], F16, kind="ExternalInput").ap()
    kvT_d = nc.dram_tensor("kvT", [BPC, KVD, M], F16, kind="ExternalInput").ap()
    wq_d = nc.dram_tensor("Wq", [QD, INNER], F16, kind="ExternalInput").ap()
    wk_d = nc.dram_tensor("Wk", [KVD, INNER], F16, kind="ExternalInput").ap()
    wv_d = nc.dram_tensor("Wv", [KVD, INNER], F16, kind="ExternalInput").ap()
    wo_d = nc.dram_tensor("Wo", [INNER, QD], F16, kind="ExternalInput").ap()
    bo_d = nc.dram_tensor("bo", [128, QD], F32, kind="ExternalInput").ap()
    y_d = nc.dram_tensor("y", [BPC, N, QD], F32, kind="ExternalOutput").ap()

    with tile.TileContext(nc) as tc, ExitStack() as ctx:
        const = ctx.enter_context(tc.tile_pool(name="const", bufs=1))
        kvt_pool = ctx.enter_context(tc.tile_pool(name="kvt", bufs=2))
        kht_pool = ctx.enter_context(tc.tile_pool(name="kht", bufs=2))
        vh_pool = ctx.enter_context(tc.tile_pool(name="vh", bufs=1))
        attn_pool = ctx.enter_context(tc.tile_pool(name="attn", bufs=16))
        outt_pool = ctx.enter_context(tc.tile_pool(name="outt", bufs=2))
        y_pool = ctx.enter_context(tc.tile_pool(name="yp", bufs=1))
        r_pool = ctx.enter_context(tc.tile_pool(name="rp", bufs=4))
        pp = ctx.enter_context(tc.tile_pool(name="pp", bufs=3, space="PSUM"))
        scp = ctx.enter_context(tc.tile_pool(name="scp", bufs=3, space="PSUM"))
        avp = ctx.enter_context(tc.tile_pool(name="avp", bufs=2, space="PSUM"))

        # ---- constants into SBUF
        wq_sb = const.tile([128, 8 * INNER], F16)
        nc.sync.dma_start(
            wq_sb[:].rearrange("p (k n) -> p k n", k=8),
            wq_d.rearrange("(k p) n -> p k n", p=128),
        )
        wk_sb = const.tile([128, 4 * INNER], F16)
        nc.sync.dma_start(
            wk_sb[:].rearrange("p (k n) -> p k n", k=4),
            wk_d.rearrange("(k p) n -> p k n", p=128),
        )
        wv_sb = const.tile([128, 4 * INNER], F16)
        nc.sync.dma_start(
            wv_sb[:].rearrange("p (k n) -> p k n", k=4),
            wv_d.rearrange("(k p) n -> p k n", p=128),
        )
        wo_sb = const.tile([128, 8 * QD], F16)
        nc.sync.dma_start(
            wo_sb[:].rearrange("p (k n) -> p k n", k=8),
            wo_d.rearrange("(k p) n -> p k n", p=128),
        )
        bo_bc = const.tile([128, QD], F32)
        nc.sync.dma_start(bo_bc[:], bo_d[:])
        ones_sb = const.tile([1, 64], F16)
        nc.vector.memset(ones_sb[:], 1.0)
        qt_sb = const.tile([128, 8 * BPC * N], F16)
        nc.sync.dma_start(
            qt_sb[:].rearrange("p (k n) -> p k n", k=8),
            qT_d.rearrange("(k p) n -> p k n", p=128),
        )

        BN = BPC * N  # 512

        # ---- qhT projection: all 4 batches at once, chunk c = head pair.
        # Layout (c, b, hi, n): per (c, b) the two head-halves sit in adjacent
        # 128-col blocks, each with the complementary 64 partition rows zeroed,
        # so ONE K=128 N=256 scores matmul serves both heads of the pair.
        qh2 = const.tile([128, 8 * 2 * BN], F16)
        qh2v = qh2[:].rearrange("p (c b i n) -> p c b i n", c=8, b=BPC, i=2)
        nc.vector.memset(qh2v[64:128, :, :, 0, :], 0.0)
        nc.vector.memset(qh2v[0:64, :, :, 1, :], 0.0)
        for c in range(8):
            p = pp.tile([128, BN], F32, tag="pp")
            for k in range(8):
                nc.tensor.matmul(
                    p[:],
                    wq_sb[:, k * INNER + c * 128:k * INNER + (c + 1) * 128],
                    qt_sb[:, k * BN:(k + 1) * BN],
                    start=(k == 0), stop=(k == 7),
                )
            nc.vector.tensor_copy(
                qh2v[0:64, c, :, 0, :],
                p[0:64, :].rearrange("p (b n) -> p b n", b=BPC),
            )
            nc.vector.tensor_copy(
                qh2v[64:128, c, :, 1, :],
                p[64:128, :].rearrange("p (b n) -> p b n", b=BPC),
            )

        vh_sb = vh_pool.tile([128, 16 * H * 65], F16, tag="vh")
        vh4 = vh_sb[:].rearrange("p (t h d) -> p t h d", h=H, d=65)
        nc.vector.memset(vh4[:, :, :, 64:65], 1.0)

        for b in range(BPC):
            # ---- load kv^T for this batch
            kvt_sb = kvt_pool.tile([128, 4 * M], F16, tag="kvt")
            nc.sync.dma_start(
                kvt_sb[:].rearrange("p (k m) -> p k m", k=4),
                kvT_d[b].rearrange("(k p) m -> p k m", p=128),
            )

            # ---- vh = kv @ Wv, kv tokens on partitions, 65-col blocks per head
            for t in range(16):
                for n2 in range(2):
                    p = pp.tile([128, 512], F32, tag="pp")
                    for k in range(4):
                        nc.tensor.matmul(
                            p[:],
                            kvt_sb[:, k * M + t * 128:k * M + (t + 1) * 128],
                            wv_sb[:, k * INNER + n2 * 512:k * INNER + (n2 + 1) * 512],
                            start=(k == 0), stop=(k == 3),
                        )
                    nc.vector.tensor_copy(
                        vh4[:, t, n2 * 8:(n2 + 1) * 8, 0:64],
                        p[:].rearrange("p (h d) -> p h d", d=64),
                    )

            # ---- per head pair: khT chunk -> scores -> exp; AV runs one
            # pair behind so the ACT exp latency hides under PE work.
            pending = None

            def do_av(avwork):
                at_list, cc = avwork
                for hi in range(2):
                    h = 2 * cc + hi
                    p = avp.tile([128, 256], F32, tag="avp")
                    for t in range(16):
                        tg, j = t // 2, t % 2
                        nc.tensor.matmul(
                            p[0:65, 0:128],
                            vh4[:, t, h, :],
                            at_list[tg][:, j * 256 + hi * 128:j * 256 + (hi + 1) * 128],
                            start=(t == 0), stop=(t == 15),
                        )
                    r32 = r_pool.tile([1, N], F32, tag="rp32")
                    nc.vector.reciprocal(r32[:], p[64:65, 0:128])
                    r16 = r_pool.tile([1, N], F16, tag="rp")
                    nc.scalar.activation(r16[:], r32[:], AF.Copy)
                    nc.tensor.matmul(p[0:64, 128:256], ones_sb[:], r16[:],
                                     start=True, stop=True)
                    r_bc = r_pool.tile([64, N], F32, tag="rbc")
                    nc.vector.tensor_copy(r_bc[:], p[0:64, 128:256])
                    dst = outt2 if hi == 0 else outt_odd
                    nc.vector.tensor_mul(
                        dst[0:64, cc * N:(cc + 1) * N],
                        p[0:64, 0:128],
                        r_bc[:],
                    )

            # Even heads' normalized outputs land directly in outt2 rows 0-63;
            # odd heads stage contiguously and one rectangular partition-shift
            # DMA folds them into rows 64-127 for the K=128 out-projection.
            outt2 = outt_pool.tile([128, 8 * N], F16, tag="outt2")
            outt_odd = outt_pool.tile([64, 8 * N], F16, tag="outt")
            for c in range(8):
                kht_sb = kht_pool.tile([128, M], F16, tag="kht")
                for n in range(4):
                    p = pp.tile([128, 512], F32, tag="pp")
                    for k in range(4):
                        nc.tensor.matmul(
                            p[:],
                            wk_sb[:, k * INNER + c * 128:k * INNER + (c + 1) * 128],
                            kvt_sb[:, k * M + n * 512:k * M + (n + 1) * 512],
                            start=(k == 0), stop=(k == 3),
                        )
                    nc.vector.tensor_copy(kht_sb[:, n * 512:(n + 1) * 512], p[:])

                at_tiles = []
                for tg in range(8):
                    sc = scp.tile([128, 512], F32, tag="scp")
                    for j in range(2):
                        t = tg * 2 + j
                        nc.tensor.matmul(
                            sc[:, j * 256:(j + 1) * 256],
                            kht_sb[:, t * 128:(t + 1) * 128],
                            qh2[:, (c * BPC + b) * 256:(c * BPC + b + 1) * 256],
                            start=True, stop=True,
                        )
                    at = attn_pool.tile([128, 512], F16, tag="attn")
                    nc.scalar.activation(at[:], sc[:], AF.Exp)
                    at_tiles.append(at)

                if pending is not None:
                    do_av(pending)
                pending = (at_tiles, c)
            do_av(pending)

            # ---- out projection: y = outT^T @ Wo + bo (K=64 per head)
            y_sb = y_pool.tile([128, QD], F32, tag="yp")
            nc.sync.dma_start(outt2[64:128, :], outt_odd[0:64, :])
            yp0 = pp.tile([128, 512], F32, tag="pp")
            yp1 = pp.tile([128, 512], F32, tag="pp")
            yps = [yp0, yp1]
            for c3 in range(8):
                for n2 in range(2):
                    nc.tensor.matmul(
                        yps[n2][:],
                        outt2[:, c3 * N:(c3 + 1) * N],
                        wo_sb[:, c3 * QD + n2 * 512:c3 * QD + (n2 + 1) * 512],
                        start=(c3 == 0), stop=(c3 == 7),
                    )
            for n2 in range(2):
                nc.vector.tensor_add(
                    y_sb[:, n2 * 512:(n2 + 1) * 512],
                    yps[n2][:],
                    bo_bc[:, n2 * 512:(n2 + 1) * 512],
                )
            nc.sync.dma_start(y_d[b], y_sb[:])

    nc.compile()
    return nc


def _get_nc():
    if "nc" not in _cached:
        _cached["nc"] = _build_nc()
    return _cached["nc"]


def kernel(q, kv, Wq, Wk, Wv, Wo, bo):
    from concourse.bass_utils import run_bass_kernel_spmd

    nc = _get_nc()

    wq16 = Wq.astype(np.float16)
    wk16 = Wk.astype(np.float16)
    wv16 = Wv.astype(np.float16)
    wo16 = Wo.astype(np.float16)
    bo32 = np.ascontiguousarray(
        np.broadcast_to(bo.reshape(1, QD), (128, QD)).astype(np.float32))

    scale = D ** -0.5  # 1/8, exact in fp16
    in_maps = []
    for i in range(NCORES):
        bs = slice(i * BPC, (i + 1) * BPC)
        # (BPC, N, QD) -> (QD, BPC, N) -> (QD, BPC*N), scale folded in
        qT = np.ascontiguousarray(
            np.transpose(q[bs] * scale, (2, 0, 1)).reshape(QD, BPC * N)
        ).astype(np.float16)
        kvT = np.ascontiguousarray(np.transpose(kv[bs], (0, 2, 1))).astype(
            np.float16
        )
        in_maps.append(
            {"qT": qT, "kvT": kvT, "Wq": wq16, "Wk": wk16, "Wv": wv16,
             "Wo": wo16, "bo": bo32}
        )

    _cached["in_maps"] = in_maps
    res = run_bass_kernel_spmd(nc, in_maps, list(range(NCORES)))
    out = np.concatenate([res.results[i]["y"] for i in range(NCORES)], axis=0)
    return out.astype(np.float32)


# revision 14
# speedup vs baseline: 1.0609x; 1.0520x over previous
"""Multi-head cross-attention (B=32, N=128, M=2048, 16 heads x 64) on 8 TRN2 cores.

Strategy: pure data-parallel over the batch dim (4 batches/core). All matmul
operands are fp16 (fp32 accumulation in PSUM); softmax skips the max-subtraction
(scores are ~N(0,1), |s|max ~ 6.5, exp stays well inside fp16 range) and the
row-sum is fused into the AV matmul as a 65th ones-column of V. Inputs are
transposed/cast on the host so every device-side matmul contracts over the
partition dim with no on-chip transposes.

Per-core device program (b = 4 batches):
  qhT  = Wq^T q^T            (heads on partitions, scale 1/8 folded into q)
  per batch:
    vh   = kv Wv             (kv tokens on partitions, + ones column per head)
    khT_c = Wk^T kv^T        (per inner chunk c of 128 = head pair)
    scoresT = khT_c^T qhT    (K=128 with the other head's q half zeroed)
    attnT = exp(scoresT)     (ACT, fp16)
    avT_h = [vh | 1]^T attnT (65 x 128; row 64 = softmax denominator)
    outT_h = avT_h / denom
    y = outT^T Wo + bo       (K=64 per head, accumulated over 16 heads)
"""
import numpy as np

NCORES = 8
B, BPC = 32, 4
N, M = 128, 2048
H, D = 16, 64
QD, KVD, INNER = 1024, 512, 1024

_cached = {}


def _build_nc():
    from contextlib import ExitStack

    import concourse.tile as tile
    from concourse import bacc, mybir

    F16 = mybir.dt.float16
    F32 = mybir.dt.float32
    AF = mybir.ActivationFunctionType

    nc = bacc.Bacc("TRN2", target_bir_lowering=False, debug=False,
                   num_devices=NCORES)
    qT_d = nc.dram_tensor("qT", [QD, BPC * N], F16, kind="ExternalInput").ap()
    kvT_d = nc.dram_tensor("kvT", [BPC, KVD, M], F16, kind="ExternalInput").ap()
    wq_d = nc.dram_tensor("Wq", [QD, INNER], F16, kind="ExternalInput").ap()
    wk_d = nc.dram_tensor("Wk", [KVD, INNER], F16, kind="ExternalInput").ap()
    wv_d = nc.dram_tensor("Wv", [KVD, INNER], F16, kind="ExternalInput").ap()
    wo_d = nc.dram_tensor("Wo", [INNER, QD], F16, kind="ExternalInput").ap()
    bo_d = nc.dram_tensor("bo", [128, QD], F32, kind="ExternalInput").ap()
    y_d = nc.dram_tensor("y", [BPC, N, QD], F32, kind="ExternalOutput").ap()

    with tile.TileContext(nc) as tc, ExitStack() as ctx:
        const = ctx.enter_context(tc.tile_pool(name="const", bufs=1))
        kvt_pool = ctx.enter_context(tc.tile_pool(name="kvt", bufs=2))
        kht_pool = ctx.enter_context(tc.tile_pool(name="kht", bufs=2))
        vh_pool = ctx.enter_context(tc.tile_pool(name="vh", bufs=1))
        attn_pool = ctx.enter_context(tc.tile_pool(name="attn", bufs=16))
        outt_pool = ctx.enter_context(tc.tile_pool(name="outt", bufs=2))
        y_pool = ctx.enter_context(tc.tile_pool(name="yp", bufs=1))
        r_pool = ctx.enter_context(tc.tile_pool(name="rp", bufs=4))
        pp = ctx.enter_context(tc.tile_pool(name="pp", bufs=3, space="PSUM"))
        scp = ctx.enter_context(tc.tile_pool(name="scp", bufs=3, space="PSUM"))
        avp = ctx.enter_context(tc.tile_pool(name="avp", bufs=2, space="PSUM"))

        # ---- constants into SBUF
        wq_sb = const.tile([128, 8 * INNER], F16)
        nc.sync.dma_start(
            wq_sb[:].rearrange("p (k n) -> p k n", k=8),
            wq_d.rearrange("(k p) n -> p k n", p=128),
        )
        wk_sb = const.tile([128, 4 * INNER], F16)
        nc.sync.dma_start(
            wk_sb[:].rearrange("p (k n) -> p k n", k=4),
            wk_d.rearrange("(k p) n -> p k n", p=128),
        )
        wv_sb = const.tile([128, 4 * INNER], F16)
        nc.sync.dma_start(
            wv_sb[:].rearrange("p (k n) -> p k n", k=4),
            wv_d.rearrange("(k p) n -> p k n", p=128),
        )
        wo_sb = const.tile([128, 8 * QD], F16)
        nc.sync.dma_start(
            wo_sb[:].rearrange("p (k n) -> p k n", k=8),
            wo_d.rearrange("(k p) n -> p k n", p=128),
        )
        bo_bc = const.tile([128, QD], F32)
        nc.sync.dma_start(bo_bc[:], bo_d[:])
        ones_sb = const.tile([1, 64], F16)
        nc.vector.memset(ones_sb[:], 1.0)
        qt_sb = const.tile([128, 8 * BPC * N], F16)
        nc.sync.dma_start(
            qt_sb[:].rearrange("p (k n) -> p k n", k=8),
            qT_d.rearrange("(k p) n -> p k n", p=128),
        )

        BN = BPC * N  # 512

        # ---- qhT projection: all 4 batches at once, chunk c = head pair.
        # Layout (c, b, hi, n): per (c, b) the two head-halves sit in adjacent
        # 128-col blocks, each with the complementary 64 partition rows zeroed,
        # so ONE K=128 N=256 scores matmul serves both heads of the pair.
        qh2 = const.tile([128, 8 * 2 * BN], F16)
        qh2v = qh2[:].rearrange("p (c b i n) -> p c b i n", c=8, b=BPC, i=2)
        nc.vector.memset(qh2v[64:128, :, :, 0, :], 0.0)
        nc.vector.memset(qh2v[0:64, :, :, 1, :], 0.0)
        for c in range(8):
            p = pp.tile([128, BN], F32, tag="pp")
            for k in range(8):
                nc.tensor.matmul(
                    p[:],
                    wq_sb[:, k * INNER + c * 128:k * INNER + (c + 1) * 128],
                    qt_sb[:, k * BN:(k + 1) * BN],
                    start=(k == 0), stop=(k == 7),
                )
            nc.vector.tensor_copy(
                qh2v[0:64, c, :, 0, :],
                p[0:64, :].rearrange("p (b n) -> p b n", b=BPC),
            )
            nc.vector.tensor_copy(
                qh2v[64:128, c, :, 1, :],
                p[64:128, :].rearrange("p (b n) -> p b n", b=BPC),
            )

        vh_sb = vh_pool.tile([128, 16 * H * 65], F16, tag="vh")
        vh4 = vh_sb[:].rearrange("p (t h d) -> p t h d", h=H, d=65)
        nc.vector.memset(vh4[:, :, :, 64:65], 1.0)

        # Out-projection for a finished batch: fold odd heads into rows
        # 64-127 (one rectangular partition-shift DMA), K=128 accumulate
        # over the 8 inner chunks, add bias, store. Deferred until after
        # the NEXT batch's vh matmuls so its serialized tail (divisions ->
        # DMA -> matmuls) hides under PE work.
        def do_outproj(work):
            bb, o2, oodd = work
            y_sb = y_pool.tile([128, QD], F32, tag="yp")
            nc.sync.dma_start(o2[64:128, :], oodd[0:64, :])
            yp0 = pp.tile([128, 512], F32, tag="pp")
            yp1 = pp.tile([128, 512], F32, tag="pp")
            yps = [yp0, yp1]
            for c3 in range(8):
                for n2 in range(2):
                    nc.tensor.matmul(
                        yps[n2][:],
                        o2[:, c3 * N:(c3 + 1) * N],
                        wo_sb[:, c3 * QD + n2 * 512:c3 * QD + (n2 + 1) * 512],
                        start=(c3 == 0), stop=(c3 == 7),
                    )
            for n2 in range(2):
                nc.vector.tensor_add(
                    y_sb[:, n2 * 512:(n2 + 1) * 512],
                    yps[n2][:],
                    bo_bc[:, n2 * 512:(n2 + 1) * 512],
                )
            nc.sync.dma_start(y_d[bb], y_sb[:])

        pending_proj = None
        for b in range(BPC):
            # ---- load kv^T for this batch
            kvt_sb = kvt_pool.tile([128, 4 * M], F16, tag="kvt")
            nc.sync.dma_start(
                kvt_sb[:].rearrange("p (k m) -> p k m", k=4),
                kvT_d[b].rearrange("(k p) m -> p k m", p=128),
            )

            # ---- vh = kv @ Wv, kv tokens on partitions, 65-col blocks per head
            for t in range(16):
                for n2 in range(2):
                    p = pp.tile([128, 512], F32, tag="pp")
                    for k in range(4):
                        nc.tensor.matmul(
                            p[:],
                            kvt_sb[:, k * M + t * 128:k * M + (t + 1) * 128],
                            wv_sb[:, k * INNER + n2 * 512:k * INNER + (n2 + 1) * 512],
                            start=(k == 0), stop=(k == 3),
                        )
                    nc.vector.tensor_copy(
                        vh4[:, t, n2 * 8:(n2 + 1) * 8, 0:64],
                        p[:].rearrange("p (h d) -> p h d", d=64),
                    )

            if pending_proj is not None:
                do_outproj(pending_proj)
                pending_proj = None

            # ---- per head pair: khT chunk -> scores -> exp; AV runs one
            # pair behind so the ACT exp latency hides under PE work.
            pending = None

            def do_av(avwork):
                at_list, cc = avwork
                for hi in range(2):
                    h = 2 * cc + hi
                    p = avp.tile([128, 256], F32, tag="avp")
                    for t in range(16):
                        tg, j = t // 2, t % 2
                        nc.tensor.matmul(
                            p[0:65, 0:128],
                            vh4[:, t, h, :],
                            at_list[tg][:, j * 256 + hi * 128:j * 256 + (hi + 1) * 128],
                            start=(t == 0), stop=(t == 15),
                        )
                    r32 = r_pool.tile([1, N], F32, tag="rp32")
                    nc.vector.reciprocal(r32[:], p[64:65, 0:128])
                    r16 = r_pool.tile([1, N], F16, tag="rp")
                    nc.scalar.activation(r16[:], r32[:], AF.Copy)
                    nc.tensor.matmul(p[0:64, 128:256], ones_sb[:], r16[:],
                                     start=True, stop=True)
                    r_bc = r_pool.tile([64, N], F32, tag="rbc")
                    nc.vector.tensor_copy(r_bc[:], p[0:64, 128:256])
                    dst = outt2 if hi == 0 else outt_odd
                    nc.vector.tensor_mul(
                        dst[0:64, cc * N:(cc + 1) * N],
                        p[0:64, 0:128],
                        r_bc[:],
                    )

            # Even heads' normalized outputs land directly in outt2 rows 0-63;
            # odd heads stage contiguously and one rectangular partition-shift
            # DMA folds them into rows 64-127 for the K=128 out-projection.
            outt2 = outt_pool.tile([128, 8 * N], F16, tag="outt2")
            outt_odd = outt_pool.tile([64, 8 * N], F16, tag="outt")
            for c in range(8):
                kht_sb = kht_pool.tile([128, M], F16, tag="kht")
                for n in range(4):
                    p = pp.tile([128, 512], F32, tag="pp")
                    for k in range(4):
                        nc.tensor.matmul(
                            p[:],
                            wk_sb[:, k * INNER + c * 128:k * INNER + (c + 1) * 128],
                            kvt_sb[:, k * M + n * 512:k * M + (n + 1) * 512],
                            start=(k == 0), stop=(k == 3),
                        )
                    nc.vector.tensor_copy(kht_sb[:, n * 512:(n + 1) * 512], p[:])

                at_tiles = []
                for tg in range(8):
                    sc = scp.tile([128, 512], F32, tag="scp")
                    for j in range(2):
                        t = tg * 2 + j
                        nc.tensor.matmul(
                            sc[:, j * 256:(j + 1) * 256],
                            kht_sb[:, t * 128:(t + 1) * 128],
                            qh2[:, (c * BPC + b) * 256:(c * BPC + b + 1) * 256],
                            start=True, stop=True,
                        )
                    at = attn_pool.tile([128, 512], F16, tag="attn")
                    nc.scalar.activation(at[:], sc[:], AF.Exp)
                    at_tiles.append(at)

                if pending is not None:
                    do_av(pending)
                pending = (at_tiles, c)
            do_av(pending)

            pending_proj = (b, outt2, outt_odd)
        do_outproj(pending_proj)

    nc.compile()
    return nc


def _get_nc():
    if "nc" not in _cached:
        _cached["nc"] = _build_nc()
    return _cached["nc"]


def kernel(q, kv, Wq, Wk, Wv, Wo, bo):
    from concourse.bass_utils import run_bass_kernel_spmd

    nc = _get_nc()

    wq16 = Wq.astype(np.float16)
    wk16 = Wk.astype(np.float16)
    wv16 = Wv.astype(np.float16)
    wo16 = Wo.astype(np.float16)
    bo32 = np.ascontiguousarray(
        np.broadcast_to(bo.reshape(1, QD), (128, QD)).astype(np.float32))

    scale = D ** -0.5  # 1/8, exact in fp16
    in_maps = []
    for i in range(NCORES):
        bs = slice(i * BPC, (i + 1) * BPC)
        # (BPC, N, QD) -> (QD, BPC, N) -> (QD, BPC*N), scale folded in
        qT = np.ascontiguousarray(
            np.transpose(q[bs] * scale, (2, 0, 1)).reshape(QD, BPC * N)
        ).astype(np.float16)
        kvT = np.ascontiguousarray(np.transpose(kv[bs], (0, 2, 1))).astype(
            np.float16
        )
        in_maps.append(
            {"qT": qT, "kvT": kvT, "Wq": wq16, "Wk": wk16, "Wv": wv16,
             "Wo": wo16, "bo": bo32}
        )

    _cached["in_maps"] = in_maps
    res = run_bass_kernel_spmd(nc, in_maps, list(range(NCORES)))
    out = np.concatenate([res.results[i]["y"] for i in range(NCORES)], axis=0)
    return out.astype(np.float32)


# revision 15
# speedup vs baseline: 1.0660x; 1.0048x over previous
"""Multi-head cross-attention (B=32, N=128, M=2048, 16 heads x 64) on 8 TRN2 cores.

Strategy: pure data-parallel over the batch dim (4 batches/core). All matmul
operands are fp16 (fp32 accumulation in PSUM); softmax skips the max-subtraction
(scores are ~N(0,1), |s|max ~ 6.5, exp stays well inside fp16 range) and the
row-sum is fused into the AV matmul as a 65th ones-column of V. Inputs are
transposed/cast on the host so every device-side matmul contracts over the
partition dim with no on-chip transposes.

Per-core device program (b = 4 batches):
  qhT  = Wq^T q^T            (heads on partitions, scale 1/8 folded into q)
  per batch:
    vh   = kv Wv             (kv tokens on partitions, + ones column per head)
    khT_c = Wk^T kv^T        (per inner chunk c of 128 = head pair)
    scoresT = khT_c^T qhT    (K=128 with the other head's q half zeroed)
    attnT = exp(scoresT)     (ACT, fp16)
    avT_h = [vh | 1]^T attnT (65 x 128; row 64 = softmax denominator)
    outT_h = avT_h / denom
    y = outT^T Wo + bo       (K=64 per head, accumulated over 16 heads)
"""
import numpy as np

NCORES = 8
B, BPC = 32, 4
N, M = 128, 2048
H, D = 16, 64
QD, KVD, INNER = 1024, 512, 1024

_cached = {}


def _build_nc():
    from contextlib import ExitStack

    import concourse.tile as tile
    from concourse import bacc, mybir

    F16 = mybir.dt.float16
    F32 = mybir.dt.float32
    AF = mybir.ActivationFunctionType

    nc = bacc.Bacc("TRN2", target_bir_lowering=False, debug=False,
                   num_devices=NCORES)
    qT_d = nc.dram_tensor("qT", [QD, BPC * N], F16, kind="ExternalInput").ap()
    kvT_d = nc.dram_tensor("kvT", [BPC, KVD, M], F16, kind="ExternalInput").ap()
    wq_d = nc.dram_tensor("Wq", [QD, INNER], F16, kind="ExternalInput").ap()
    wk_d = nc.dram_tensor("Wk", [KVD, INNER], F16, kind="ExternalInput").ap()
    wv_d = nc.dram_tensor("Wv", [KVD, INNER], F16, kind="ExternalInput").ap()
    wo_d = nc.dram_tensor("Wo", [INNER, QD], F16, kind="ExternalInput").ap()
    bo_d = nc.dram_tensor("bo", [128, QD], F32, kind="ExternalInput").ap()
    y_d = nc.dram_tensor("y", [BPC, N, QD], F32, kind="ExternalOutput").ap()

    with tile.TileContext(nc) as tc, ExitStack() as ctx:
        const = ctx.enter_context(tc.tile_pool(name="const", bufs=1))
        kvt_pool = ctx.enter_context(tc.tile_pool(name="kvt", bufs=2))
        kht_pool = ctx.enter_context(tc.tile_pool(name="kht", bufs=3))
        vh_pool = ctx.enter_context(tc.tile_pool(name="vh", bufs=1))
        attn_pool = ctx.enter_context(tc.tile_pool(name="attn", bufs=20))
        outt_pool = ctx.enter_context(tc.tile_pool(name="outt", bufs=2))
        y_pool = ctx.enter_context(tc.tile_pool(name="yp", bufs=1))
        r_pool = ctx.enter_context(tc.tile_pool(name="rp", bufs=4))
        pp = ctx.enter_context(tc.tile_pool(name="pp", bufs=3, space="PSUM"))
        scp = ctx.enter_context(tc.tile_pool(name="scp", bufs=3, space="PSUM"))
        avp = ctx.enter_context(tc.tile_pool(name="avp", bufs=2, space="PSUM"))

        # ---- constants into SBUF
        wq_sb = const.tile([128, 8 * INNER], F16)
        nc.sync.dma_start(
            wq_sb[:].rearrange("p (k n) -> p k n", k=8),
            wq_d.rearrange("(k p) n -> p k n", p=128),
        )
        wk_sb = const.tile([128, 4 * INNER], F16)
        nc.sync.dma_start(
            wk_sb[:].rearrange("p (k n) -> p k n", k=4),
            wk_d.rearrange("(k p) n -> p k n", p=128),
        )
        wv_sb = const.tile([128, 4 * INNER], F16)
        nc.sync.dma_start(
            wv_sb[:].rearrange("p (k n) -> p k n", k=4),
            wv_d.rearrange("(k p) n -> p k n", p=128),
        )
        wo_sb = const.tile([128, 8 * QD], F16)
        nc.sync.dma_start(
            wo_sb[:].rearrange("p (k n) -> p k n", k=8),
            wo_d.rearrange("(k p) n -> p k n", p=128),
        )
        bo_bc = const.tile([128, QD], F32)
        nc.sync.dma_start(bo_bc[:], bo_d[:])
        ones_sb = const.tile([1, 64], F16)
        nc.vector.memset(ones_sb[:], 1.0)
        qt_sb = const.tile([128, 8 * BPC * N], F16)
        nc.sync.dma_start(
            qt_sb[:].rearrange("p (k n) -> p k n", k=8),
            qT_d.rearrange("(k p) n -> p k n", p=128),
        )

        BN = BPC * N  # 512

        # ---- qhT projection: all 4 batches at once, chunk c = head pair.
        # Layout (c, b, hi, n): per (c, b) the two head-halves sit in adjacent
        # 128-col blocks, each with the complementary 64 partition rows zeroed,
        # so ONE K=128 N=256 scores matmul serves both heads of the pair.
        qh2 = const.tile([128, 8 * 2 * BN], F16)
        qh2v = qh2[:].rearrange("p (c b i n) -> p c b i n", c=8, b=BPC, i=2)
        nc.vector.memset(qh2v[64:128, :, :, 0, :], 0.0)
        nc.vector.memset(qh2v[0:64, :, :, 1, :], 0.0)
        for c in range(8):
            p = pp.tile([128, BN], F32, tag="pp")
            for k in range(8):
                nc.tensor.matmul(
                    p[:],
                    wq_sb[:, k * INNER + c * 128:k * INNER + (c + 1) * 128],
                    qt_sb[:, k * BN:(k + 1) * BN],
                    start=(k == 0), stop=(k == 7),
                )
            nc.vector.tensor_copy(
                qh2v[0:64, c, :, 0, :],
                p[0:64, :].rearrange("p (b n) -> p b n", b=BPC),
            )
            nc.vector.tensor_copy(
                qh2v[64:128, c, :, 1, :],
                p[64:128, :].rearrange("p (b n) -> p b n", b=BPC),
            )

        vh_sb = vh_pool.tile([128, 16 * H * 65], F16, tag="vh")
        vh4 = vh_sb[:].rearrange("p (t h d) -> p t h d", h=H, d=65)
        nc.vector.memset(vh4[:, :, :, 64:65], 1.0)

        # Out-projection for a finished batch: fold odd heads into rows
        # 64-127 (one rectangular partition-shift DMA), K=128 accumulate
        # over the 8 inner chunks, add bias, store. Deferred until after
        # the NEXT batch's vh matmuls so its serialized tail (divisions ->
        # DMA -> matmuls) hides under PE work.
        def do_outproj(work):
            bb, o2, oodd = work
            y_sb = y_pool.tile([128, QD], F32, tag="yp")
            nc.sync.dma_start(o2[64:128, :], oodd[0:64, :])
            yp0 = pp.tile([128, 512], F32, tag="pp")
            yp1 = pp.tile([128, 512], F32, tag="pp")
            yps = [yp0, yp1]
            for c3 in range(8):
                for n2 in range(2):
                    nc.tensor.matmul(
                        yps[n2][:],
                        o2[:, c3 * N:(c3 + 1) * N],
                        wo_sb[:, c3 * QD + n2 * 512:c3 * QD + (n2 + 1) * 512],
                        start=(c3 == 0), stop=(c3 == 7),
                    )
            for n2 in range(2):
                nc.vector.tensor_add(
                    y_sb[:, n2 * 512:(n2 + 1) * 512],
                    yps[n2][:],
                    bo_bc[:, n2 * 512:(n2 + 1) * 512],
                )
            nc.sync.dma_start(y_d[bb], y_sb[:])

        pending_proj = None
        for b in range(BPC):
            # ---- load kv^T for this batch
            kvt_sb = kvt_pool.tile([128, 4 * M], F16, tag="kvt")
            nc.sync.dma_start(
                kvt_sb[:].rearrange("p (k m) -> p k m", k=4),
                kvT_d[b].rearrange("(k p) m -> p k m", p=128),
            )

            # ---- vh = kv @ Wv, kv tokens on partitions, 65-col blocks per head
            for t in range(16):
                for n2 in range(2):
                    p = pp.tile([128, 512], F32, tag="pp")
                    for k in range(4):
                        nc.tensor.matmul(
                            p[:],
                            kvt_sb[:, k * M + t * 128:k * M + (t + 1) * 128],
                            wv_sb[:, k * INNER + n2 * 512:k * INNER + (n2 + 1) * 512],
                            start=(k == 0), stop=(k == 3),
                        )
                    nc.vector.tensor_copy(
                        vh4[:, t, n2 * 8:(n2 + 1) * 8, 0:64],
                        p[:].rearrange("p (h d) -> p h d", d=64),
                    )

            if pending_proj is not None:
                do_outproj(pending_proj)
                pending_proj = None

            # ---- per head pair: khT chunk -> scores -> exp; AV runs one
            # pair behind so the ACT exp latency hides under PE work.
            pending = None

            def do_av(avwork):
                at_list, cc = avwork
                for hi in range(2):
                    h = 2 * cc + hi
                    p = avp.tile([128, 256], F32, tag="avp")
                    for t in range(16):
                        tg, j = t // 2, t % 2
                        nc.tensor.matmul(
                            p[0:65, 0:128],
                            vh4[:, t, h, :],
                            at_list[tg][:, j * 256 + hi * 128:j * 256 + (hi + 1) * 128],
                            start=(t == 0), stop=(t == 15),
                        )
                    r32 = r_pool.tile([1, N], F32, tag="rp32")
                    nc.vector.reciprocal(r32[:], p[64:65, 0:128])
                    r16 = r_pool.tile([1, N], F16, tag="rp")
                    nc.scalar.activation(r16[:], r32[:], AF.Copy)
                    nc.tensor.matmul(p[0:64, 128:256], ones_sb[:], r16[:],
                                     start=True, stop=True)
                    r_bc = r_pool.tile([64, N], F32, tag="rbc")
                    nc.vector.tensor_copy(r_bc[:], p[0:64, 128:256])
                    dst = outt2 if hi == 0 else outt_odd
                    nc.vector.tensor_mul(
                        dst[0:64, cc * N:(cc + 1) * N],
                        p[0:64, 0:128],
                        r_bc[:],
                    )

            # Even heads' normalized outputs land directly in outt2 rows 0-63;
            # odd heads stage contiguously and one rectangular partition-shift
            # DMA folds them into rows 64-127 for the K=128 out-projection.
            outt2 = outt_pool.tile([128, 8 * N], F16, tag="outt2")
            outt_odd = outt_pool.tile([64, 8 * N], F16, tag="outt")
            for c in range(8):
                kht_sb = kht_pool.tile([128, M], F16, tag="kht")
                for n in range(4):
                    p = pp.tile([128, 512], F32, tag="pp")
                    for k in range(4):
                        nc.tensor.matmul(
                            p[:],
                            wk_sb[:, k * INNER + c * 128:k * INNER + (c + 1) * 128],
                            kvt_sb[:, k * M + n * 512:k * M + (n + 1) * 512],
                            start=(k == 0), stop=(k == 3),
                        )
                    nc.vector.tensor_copy(kht_sb[:, n * 512:(n + 1) * 512], p[:])

                at_tiles = []
                for tg in range(8):
                    sc = scp.tile([128, 512], F32, tag="scp")
                    for j in range(2):
                        t = tg * 2 + j
                        nc.tensor.matmul(
                            sc[:, j * 256:(j + 1) * 256],
                            kht_sb[:, t * 128:(t + 1) * 128],
                            qh2[:, (c * BPC + b) * 256:(c * BPC + b + 1) * 256],
                            start=True, stop=True,
                        )
                    at = attn_pool.tile([128, 512], F16, tag="attn")
                    nc.scalar.activation(at[:], sc[:], AF.Exp)
                    at_tiles.append(at)

                if pending is not None:
                    do_av(pending)
                pending = (at_tiles, c)
            do_av(pending)

            pending_proj = (b, outt2, outt_odd)
        do_outproj(pending_proj)

    nc.compile()
    return nc


def _get_nc():
    if "nc" not in _cached:
        _cached["nc"] = _build_nc()
    return _cached["nc"]


def kernel(q, kv, Wq, Wk, Wv, Wo, bo):
    from concourse.bass_utils import run_bass_kernel_spmd

    nc = _get_nc()

    wq16 = Wq.astype(np.float16)
    wk16 = Wk.astype(np.float16)
    wv16 = Wv.astype(np.float16)
    wo16 = Wo.astype(np.float16)
    bo32 = np.ascontiguousarray(
        np.broadcast_to(bo.reshape(1, QD), (128, QD)).astype(np.float32))

    scale = D ** -0.5  # 1/8, exact in fp16
    in_maps = []
    for i in range(NCORES):
        bs = slice(i * BPC, (i + 1) * BPC)
        # (BPC, N, QD) -> (QD, BPC, N) -> (QD, BPC*N), scale folded in
        qT = np.ascontiguousarray(
            np.transpose(q[bs] * scale, (2, 0, 1)).reshape(QD, BPC * N)
        ).astype(np.float16)
        kvT = np.ascontiguousarray(np.transpose(kv[bs], (0, 2, 1))).astype(
            np.float16
        )
        in_maps.append(
            {"qT": qT, "kvT": kvT, "Wq": wq16, "Wk": wk16, "Wv": wv16,
             "Wo": wo16, "bo": bo32}
        )

    _cached["in_maps"] = in_maps
    res = run_bass_kernel_spmd(nc, in_maps, list(range(NCORES)))
    out = np.concatenate([res.results[i]["y"] for i in range(NCORES)], axis=0)
    return out.astype(np.float32)
